# revision 5
# baseline (speedup 1.0000x reference)
"""Trainium2 Bass kernel for nn_Attention (dense transformer cross-attention).

Strategy: data-parallel over batch (B=8) -> one batch element per NeuronCore.
Per core, everything is computed with zero on-chip transposes by choosing
layouts up front (host pre-transposes activations/weights, which is free):

  K^T_h [dh=128, M]   = Wk-chunk^T . memory^T   (per head h, bias via ACT)
  Q^T_h [dh=128, Q]   = (scaled Wq)-chunk^T . query^T
  V     [M, D]        = memory . Wv^T + bv      (natural layout)
  S^T   [M, Q]        = K^T_h^T-free-slices . Q^T_h    (heads pipelined)
  expS  = ACT Exp with per-partition mask bias (-1e30 -> exact 0), bf16
  sum_q = ones-row matmul over expS (partition-direction sum on PE)
  ctx^T_h [dh, Q]     = V-chunks . expS  (PSUM accum over m-chunks)
  normalization       = 1/sum broadcast via DRAM-bounce DMA, fused into
                        the ctx PSUM->SBUF drain (DVE multiply)
  out   [Q, D]        = ctx^T (as lhsT, heads = contraction chunks) . Wf^T
                        + bf (fused into final drain)

Softmax max-subtraction is skipped: scores are O(1) by construction
(0.02-scale weights), exp is computed in f32 on ACT, so this is exact.

Compute dtype bf16 (f32 PSUM accumulation); inputs converted on host so
device DMA moves half the bytes.
"""

import math

import numpy as np
import ml_dtypes

B = 8
Q = 1024
M = 1024
D = 1024
H = 8
DH = 128
KC = 8  # 128-row contraction chunks per 1024
NT = 2  # 512-wide free tiles per 1024
FT = 512

_BF16 = ml_dtypes.bfloat16
_CACHE = {}


def _build_program():
    import concourse.bass as bass
    import concourse.mybir as mybir
    from concourse.tile import TileContext
    from concourse.vector_clock import ScopedClock

    import bass_rust

    f32 = mybir.dt.float32
    bf16 = mybir.dt.bfloat16
    Identity = mybir.ActivationFunctionType.Identity
    Exp = mybir.ActivationFunctionType.Exp

    def split_sync_waits(nc):
        """The walrus in this container accepts only ONE sync-wait per
        instruction; Tile freely attaches several. Move excess waits onto
        same-engine NOPs spliced immediately before the instruction."""
        for fn in nc.m.functions:
            for bb in fn.blocks:
                out = []
                for inst in bb.instructions:
                    si = inst.sync_info
                    if si is not None and si.on_wait is not None and len(si.on_wait) > 1:
                        waits = list(si.on_wait)
                        si.on_wait = waits[-1:]
                        for j, w in enumerate(waits[:-1]):
                            nop = bass_rust.InstNoOp(
                                name=f"{inst.name}_sw{j}", ins=[], outs=[])
                            nop.engine = inst.engine
                            nop.sync_info = mybir.SyncInfo(on_wait=[w], on_update=[])
                            out.append(nop)
                    out.append(inst)
                bb.instructions = out

    nc = bass.Bass()

    memT = nc.declare_dram_parameter("memT", [D, M], bf16, isOutput=False)
    qT = nc.declare_dram_parameter("qT", [D, Q], bf16, isOutput=False)
    wkT = nc.declare_dram_parameter("wkT", [D, D], bf16, isOutput=False)
    wvT = nc.declare_dram_parameter("wvT", [D, D], bf16, isOutput=False)
    wqT = nc.declare_dram_parameter("wqT", [D, D], bf16, isOutput=False)
    wfT = nc.declare_dram_parameter("wfT", [D, D], bf16, isOutput=False)
    bk_pp = nc.declare_dram_parameter("bk_pp", [128, H], f32, isOutput=False)
    bq_pp = nc.declare_dram_parameter("bq_pp", [128, H], f32, isOutput=False)
    mb_pp = nc.declare_dram_parameter("mb_pp", [128, KC], f32, isOutput=False)
    bv_bc = nc.declare_dram_parameter("bv_bc", [128, D], f32, isOutput=False)
    bf_bc = nc.declare_dram_parameter("bf_bc", [128, D], f32, isOutput=False)

    wm = nc.declare_dram_parameter("wm", [Q, D], f32, isOutput=True)
    p0t = nc.declare_dram_parameter("p0t", [M, Q], f32, isOutput=True)

    def chunked(dram_ap):
        # [1024, N] DRAM -> [p=128, c=8, N] access pattern
        return dram_ap.rearrange("(c p) n -> p c n", p=128)

    with TileContext(nc) as tc:
        with tc.tile_pool(name="const", bufs=1) as const, \
             tc.tile_pool(name="persist", bufs=1) as persist:
            wf_sb = const.tile([128, KC, D], bf16)
            bkt = const.tile([128, H], f32)
            bqt = const.tile([128, H], f32)
            mbt = const.tile([128, KC], f32)
            bvt = const.tile([128, D], f32)
            bft = const.tile([128, D], f32)
            ones_col = const.tile([128, 1], bf16)

            nc.sync.dma_start(out=wf_sb[:], in_=chunked(wfT[:, :]))
            nc.sync.dma_start(out=bkt[:], in_=bk_pp[:, :])
            nc.sync.dma_start(out=bqt[:], in_=bq_pp[:, :])
            nc.sync.dma_start(out=mbt[:], in_=mb_pp[:, :])
            nc.sync.dma_start(out=bvt[:], in_=bv_bc[:, :])
            nc.sync.dma_start(out=bft[:], in_=bf_bc[:, :])
            nc.vector.memset(ones_col[:], 1.0)

            k_sb = persist.tile([128, H, M], bf16)
            q_sb = persist.tile([128, H, Q], bf16)
            v_sb = persist.tile([128, KC, D], bf16)
            ctx_sb = persist.tile([128, H, Q], bf16)

            # ---------------- Phase 1: projections ----------------
            with tc.tile_pool(name="proj", bufs=1) as proj, \
                 tc.tile_pool(name="ppsum", bufs=4, space="PSUM") as ppsum:
                mem_sb = proj.tile([128, KC, M], bf16)
                qt_sb = proj.tile([128, KC, Q], bf16)
                wk_sb = proj.tile([128, KC, D], bf16)
                wv_sb = proj.tile([128, KC, D], bf16)
                wq_sb = proj.tile([128, KC, D], bf16)
                nc.sync.dma_start(out=mem_sb[:], in_=chunked(memT[:, :]))
                nc.sync.dma_start(out=qt_sb[:], in_=chunked(qT[:, :]))
                nc.sync.dma_start(out=wk_sb[:], in_=chunked(wkT[:, :]))
                nc.sync.dma_start(out=wv_sb[:], in_=chunked(wvT[:, :]))
                nc.sync.dma_start(out=wq_sb[:], in_=chunked(wqT[:, :]))

                for h in range(H):
                    hs = slice(h * DH, (h + 1) * DH)
                    for t in range(NT):
                        ts_ = slice(t * FT, (t + 1) * FT)
                        ps = ppsum.tile([128, FT], f32, tag="pp")
                        for c in range(KC):
                            nc.tensor.matmul(
                                ps[:], wk_sb[:, c, hs], mem_sb[:, c, ts_],
                                start=(c == 0), stop=(c == KC - 1))
                        nc.scalar.activation(
                            k_sb[:, h, ts_], ps[:], Identity, bias=bkt[:, h:h + 1])
                        ps2 = ppsum.tile([128, FT], f32, tag="pp")
                        for c in range(KC):
                            nc.tensor.matmul(
                                ps2[:], wq_sb[:, c, hs], qt_sb[:, c, ts_],
                                start=(c == 0), stop=(c == KC - 1))
                        nc.scalar.activation(
                            q_sb[:, h, ts_], ps2[:], Identity, bias=bqt[:, h:h + 1])

                for mc in range(KC):
                    ms = slice(mc * DH, (mc + 1) * DH)
                    for t in range(NT):
                        ts_ = slice(t * FT, (t + 1) * FT)
                        ps = ppsum.tile([128, FT], f32, tag="pp")
                        for c in range(KC):
                            nc.tensor.matmul(
                                ps[:], mem_sb[:, c, ms], wv_sb[:, c, ts_],
                                start=(c == 0), stop=(c == KC - 1))
                        nc.vector.tensor_add(v_sb[:, mc, ts_], ps[:], bvt[:, ts_])

            # ---------------- Phase 2: attention (per head) ----------------
            with tc.tile_pool(name="attn", bufs=2) as attn, \
                 tc.tile_pool(name="attn3", bufs=3) as attn3, \
                 tc.tile_pool(name="dramp", bufs=2, space="DRAM") as dramp, \
                 tc.tile_pool(name="spsum", bufs=3, space="PSUM") as spsum, \
                 tc.tile_pool(name="cpsum", bufs=2, space="PSUM") as cpsum, \
                 tc.tile_pool(name="upsum", bufs=2, space="PSUM") as upsum:
                for h in range(H):
                    hs = slice(h * DH, (h + 1) * DH)
                    exp_sb = attn.tile([128, KC, Q], bf16, tag="expS")
                    sum_ps = [upsum.tile([1, FT], f32, tag="sum", name=f"sum_h{h}_t{t}")
                              for t in range(NT)]
                    for c in range(KC):
                        cs = slice(c * DH, (c + 1) * DH)
                        for t in range(NT):
                            ts_ = slice(t * FT, (t + 1) * FT)
                            st = spsum.tile([128, FT], f32, tag="st")
                            nc.tensor.matmul(
                                st[:], k_sb[:, h, cs], q_sb[:, h, ts_],
                                start=True, stop=True)
                            nc.scalar.activation(
                                exp_sb[:, c, ts_], st[:], Exp, bias=mbt[:, c:c + 1])
                            nc.tensor.matmul(
                                sum_ps[t][:], ones_col[:], exp_sb[:, c, ts_],
                                start=(c == 0), stop=(c == KC - 1))
                    srow = attn.tile([1, Q], f32, tag="srow")
                    for t in range(NT):
                        nc.vector.tensor_copy(
                            srow[0:1, t * FT:(t + 1) * FT], sum_ps[t][:])
                    rrow = attn.tile([1, Q], f32, tag="rrow")
                    nc.vector.reciprocal(rrow[:], srow[:])
                    rcp_d = dramp.tile([1, Q], f32, tag="rcpd")
                    nc.sync.dma_start(out=rcp_d[:], in_=rrow[:])
                    rb_sb = attn.tile([128, Q], f32, tag="rb")
                    nc.sync.dma_start(out=rb_sb[:], in_=rcp_d[0:1, :].to_broadcast([128, Q]))

                    for t in range(NT):
                        ts_ = slice(t * FT, (t + 1) * FT)
                        cp = cpsum.tile([128, FT], f32, tag="cp")
                        for c in range(KC):
                            nc.tensor.matmul(
                                cp[:], v_sb[:, c, hs], exp_sb[:, c, ts_],
                                start=(c == 0), stop=(c == KC - 1))
                        nc.vector.tensor_mul(ctx_sb[:, h, ts_], cp[:], rb_sb[:, ts_])

                    if h == 0:
                        for c in range(KC):
                            p0_sb = attn3.tile([128, Q], f32, tag="p0")
                            nc.vector.tensor_mul(p0_sb[:], exp_sb[:, c, :], rb_sb[:])
                            nc.sync.dma_start(
                                out=p0t[c * DH:(c + 1) * DH, :], in_=p0_sb[:])

            # ---------------- Phase 3: final projection ----------------
            with tc.tile_pool(name="fin", bufs=3) as fin, \
                 tc.tile_pool(name="fpsum", bufs=3, space="PSUM") as fpsum:
                for qc in range(KC):
                    qs = slice(qc * DH, (qc + 1) * DH)
                    for t in range(NT):
                        ts_ = slice(t * FT, (t + 1) * FT)
                        fp = fpsum.tile([128, FT], f32, tag="fp")
                        for h in range(H):
                            nc.tensor.matmul(
                                fp[:], ctx_sb[:, h, qs], wf_sb[:, h, ts_],
                                start=(h == 0), stop=(h == H - 1))
                        of = fin.tile([128, FT], f32, tag="of")
                        nc.vector.tensor_add(of[:], fp[:], bft[:, ts_])
                        nc.sync.dma_start(out=wm[qs, ts_], in_=of[:])

    split_sync_waits(nc)
    return nc


def _get_program():
    if "nc" not in _CACHE:
        _CACHE["nc"] = _build_program()
    return _CACHE["nc"]


def _host_prep(query, memory, mask, Wk, bk, Wv, bv, Wq, bq, Wf, bf):
    scale = 1.0 / math.sqrt(DH)
    f32 = np.float32

    def t_bf16(a):
        return np.ascontiguousarray(np.asarray(a, dtype=f32).T).astype(_BF16)

    shared = {
        "wkT": t_bf16(Wk),
        "wvT": t_bf16(Wv),
        "wqT": np.ascontiguousarray(
            np.asarray(Wq, dtype=f32).T * f32(scale)).astype(_BF16),
        "wfT": t_bf16(Wf),
        "bk_pp": np.ascontiguousarray(
            np.asarray(bk, dtype=f32).reshape(H, DH).T),
        "bq_pp": np.ascontiguousarray(
            (np.asarray(bq, dtype=f32) * f32(scale)).reshape(H, DH).T),
        "bv_bc": np.ascontiguousarray(
            np.broadcast_to(np.asarray(bv, dtype=f32), (128, D))),
        "bf_bc": np.ascontiguousarray(
            np.broadcast_to(np.asarray(bf, dtype=f32), (128, D))),
    }
    mask = np.asarray(mask)
    in_maps = []
    for b in range(B):
        mb = np.where(mask[b], f32(-1e30), f32(0.0)).astype(f32)
        in_maps.append({
            **shared,
            "memT": t_bf16(memory[b]),
            "qT": t_bf16(query[b]),
            "mb_pp": np.ascontiguousarray(mb.reshape(KC, DH).T),
        })
    return in_maps


def kernel(query, memory, mask, Wk, bk, Wv, bv, Wq, bq, Wf, bf):
    from concourse.bass_utils import run_bass_kernel_spmd

    nc = _get_program()
    in_maps = _host_prep(query, memory, mask, Wk, bk, Wv, bv, Wq, bq, Wf, bf)
    res = run_bass_kernel_spmd(nc, in_maps, core_ids=list(range(B)))
    wm = np.stack([res.results[b]["wm"] for b in range(B)])
    w0 = np.stack([np.ascontiguousarray(res.results[b]["p0t"].T)
                   for b in range(B)])
    return wm.astype(np.float32), w0.astype(np.float32)


# revision 6
# speedup vs baseline: 237.8964x; 237.8964x over previous
"""Trainium2 Bass kernel for nn_Attention (dense transformer cross-attention).

Strategy: data-parallel over batch (B=8) -> one batch element per NeuronCore.
Per core, everything is computed with zero on-chip transposes by choosing
layouts up front (host pre-transposes activations/weights, which is free):

  K^T_h [dh=128, M]   = Wk-chunk^T . memory^T   (per head h, bias via ACT)
  Q^T_h [dh=128, Q]   = (scaled Wq)-chunk^T . query^T
  V     [M, D]        = memory . Wv^T + bv      (natural layout)
  S^T   [M, Q]        = K^T_h^T-free-slices . Q^T_h    (heads pipelined)
  expS  = ACT Exp with per-partition mask bias (-1e30 -> exact 0), bf16
  sum_q = ones-row matmul over expS (partition-direction sum on PE)
  ctx^T_h [dh, Q]     = V-chunks . expS  (PSUM accum over m-chunks)
  normalization       = 1/sum broadcast via DRAM-bounce DMA, fused into
                        the ctx PSUM->SBUF drain (DVE multiply)
  out   [Q, D]        = ctx^T (as lhsT, heads = contraction chunks) . Wf^T
                        + bf (fused into final drain)

Softmax max-subtraction is skipped: scores are O(1) by construction
(0.02-scale weights), exp is computed in f32 on ACT, so this is exact.

Compute dtype bf16 (f32 PSUM accumulation); inputs converted on host so
device DMA moves half the bytes.
"""

import math

import numpy as np
import ml_dtypes

B = 8
Q = 1024
M = 1024
D = 1024
H = 8
DH = 128
KC = 8  # 128-row contraction chunks per 1024
NT = 2  # 512-wide free tiles per 1024
FT = 512

_BF16 = ml_dtypes.bfloat16
_CACHE = {}


def _build_program(reps=1):
    import concourse.bass as bass
    import concourse.mybir as mybir
    from concourse.tile import TileContext
    from concourse.vector_clock import ScopedClock

    import bass_rust

    f32 = mybir.dt.float32
    bf16 = mybir.dt.bfloat16
    Identity = mybir.ActivationFunctionType.Identity
    Exp = mybir.ActivationFunctionType.Exp

    def split_sync_waits(nc):
        """The walrus in this container accepts only ONE sync-wait per
        instruction; Tile freely attaches several. Move excess waits onto
        same-engine NOPs spliced immediately before the instruction."""
        for fn in nc.m.functions:
            for bb in fn.blocks:
                out = []
                for inst in bb.instructions:
                    si = inst.sync_info
                    if si is not None and si.on_wait is not None and len(si.on_wait) > 1:
                        waits = list(si.on_wait)
                        si.on_wait = waits[-1:]
                        for j, w in enumerate(waits[:-1]):
                            nop = bass_rust.InstNoOp(
                                name=f"{inst.name}_sw{j}", ins=[], outs=[])
                            nop.engine = inst.engine
                            nop.sync_info = mybir.SyncInfo(on_wait=[w], on_update=[])
                            out.append(nop)
                    out.append(inst)
                bb.instructions = out

    nc = bass.Bass()

    memT = nc.declare_dram_parameter("memT", [D, M], bf16, isOutput=False)
    qT = nc.declare_dram_parameter("qT", [D, Q], bf16, isOutput=False)
    wkT = nc.declare_dram_parameter("wkT", [D, D], bf16, isOutput=False)
    wvT = nc.declare_dram_parameter("wvT", [D, D], bf16, isOutput=False)
    wqT = nc.declare_dram_parameter("wqT", [D, D], bf16, isOutput=False)
    wfT = nc.declare_dram_parameter("wfT", [D, D], bf16, isOutput=False)
    bk_pp = nc.declare_dram_parameter("bk_pp", [128, H], f32, isOutput=False)
    bq_pp = nc.declare_dram_parameter("bq_pp", [128, H], f32, isOutput=False)
    mb_pp = nc.declare_dram_parameter("mb_pp", [128, KC], f32, isOutput=False)
    bv_bc = nc.declare_dram_parameter("bv_bc", [128, D], f32, isOutput=False)
    bf_bc = nc.declare_dram_parameter("bf_bc", [128, D], f32, isOutput=False)

    wm = nc.declare_dram_parameter("wm", [Q, D], f32, isOutput=True)
    p0t = nc.declare_dram_parameter("p0t", [M, Q], f32, isOutput=True)

    def chunked(dram_ap):
        # [1024, N] DRAM -> [p=128, c=8, N] access pattern
        return dram_ap.rearrange("(c p) n -> p c n", p=128)

    with TileContext(nc) as tc:
      for rep in range(reps):
        with tc.tile_pool(name=f"const{rep}", bufs=1) as const, \
             tc.tile_pool(name=f"persist{rep}", bufs=1) as persist:
            wf_sb = const.tile([128, KC, D], bf16)
            bkt = const.tile([128, H], f32)
            bqt = const.tile([128, H], f32)
            mbt = const.tile([128, KC], f32)
            bvt = const.tile([128, D], f32)
            bft = const.tile([128, D], f32)
            ones_col = const.tile([128, 1], bf16)

            nc.sync.dma_start(out=wf_sb[:], in_=chunked(wfT[:, :]))
            nc.sync.dma_start(out=bkt[:], in_=bk_pp[:, :])
            nc.sync.dma_start(out=bqt[:], in_=bq_pp[:, :])
            nc.sync.dma_start(out=mbt[:], in_=mb_pp[:, :])
            nc.sync.dma_start(out=bvt[:], in_=bv_bc[:, :])
            nc.sync.dma_start(out=bft[:], in_=bf_bc[:, :])
            nc.vector.memset(ones_col[:], 1.0)

            k_sb = persist.tile([128, H, M], bf16)
            q_sb = persist.tile([128, H, Q], bf16)
            v_sb = persist.tile([128, KC, D], bf16)
            ctx_sb = persist.tile([128, H, Q], bf16)

            # ---------------- Phase 1: projections ----------------
            with tc.tile_pool(name=f"proj{rep}", bufs=1) as proj, \
                 tc.tile_pool(name=f"ppsum{rep}", bufs=4, space="PSUM") as ppsum:
                mem_sb = proj.tile([128, KC, M], bf16)
                qt_sb = proj.tile([128, KC, Q], bf16)
                wk_sb = proj.tile([128, KC, D], bf16)
                wv_sb = proj.tile([128, KC, D], bf16)
                wq_sb = proj.tile([128, KC, D], bf16)
                nc.sync.dma_start(out=mem_sb[:], in_=chunked(memT[:, :]))
                nc.sync.dma_start(out=qt_sb[:], in_=chunked(qT[:, :]))
                nc.sync.dma_start(out=wk_sb[:], in_=chunked(wkT[:, :]))
                nc.sync.dma_start(out=wv_sb[:], in_=chunked(wvT[:, :]))
                nc.sync.dma_start(out=wq_sb[:], in_=chunked(wqT[:, :]))

                for h in range(H):
                    hs = slice(h * DH, (h + 1) * DH)
                    for t in range(NT):
                        ts_ = slice(t * FT, (t + 1) * FT)
                        ps = ppsum.tile([128, FT], f32, tag="pp")
                        for c in range(KC):
                            nc.tensor.matmul(
                                ps[:], wk_sb[:, c, hs], mem_sb[:, c, ts_],
                                start=(c == 0), stop=(c == KC - 1))
                        nc.scalar.activation(
                            k_sb[:, h, ts_], ps[:], Identity, bias=bkt[:, h:h + 1])
                        ps2 = ppsum.tile([128, FT], f32, tag="pp")
                        for c in range(KC):
                            nc.tensor.matmul(
                                ps2[:], wq_sb[:, c, hs], qt_sb[:, c, ts_],
                                start=(c == 0), stop=(c == KC - 1))
                        nc.scalar.activation(
                            q_sb[:, h, ts_], ps2[:], Identity, bias=bqt[:, h:h + 1])

                for mc in range(KC):
                    ms = slice(mc * DH, (mc + 1) * DH)
                    for t in range(NT):
                        ts_ = slice(t * FT, (t + 1) * FT)
                        ps = ppsum.tile([128, FT], f32, tag="pp")
                        for c in range(KC):
                            nc.tensor.matmul(
                                ps[:], mem_sb[:, c, ms], wv_sb[:, c, ts_],
                                start=(c == 0), stop=(c == KC - 1))
                        nc.vector.tensor_add(v_sb[:, mc, ts_], ps[:], bvt[:, ts_])

            # ---------------- Phase 2: attention (per head) ----------------
            with tc.tile_pool(name=f"attn{rep}", bufs=2) as attn, \
                 tc.tile_pool(name=f"attn3{rep}", bufs=3) as attn3, \
                 tc.tile_pool(name=f"dramp{rep}", bufs=2, space="DRAM") as dramp, \
                 tc.tile_pool(name=f"spsum{rep}", bufs=3, space="PSUM") as spsum, \
                 tc.tile_pool(name=f"cpsum{rep}", bufs=2, space="PSUM") as cpsum, \
                 tc.tile_pool(name=f"upsum{rep}", bufs=2, space="PSUM") as upsum:
                for h in range(H):
                    hs = slice(h * DH, (h + 1) * DH)
                    exp_sb = attn.tile([128, KC, Q], bf16, tag="expS")
                    sum_ps = [upsum.tile([1, FT], f32, tag="sum", name=f"sum_h{h}_t{t}")
                              for t in range(NT)]
                    for c in range(KC):
                        cs = slice(c * DH, (c + 1) * DH)
                        for t in range(NT):
                            ts_ = slice(t * FT, (t + 1) * FT)
                            st = spsum.tile([128, FT], f32, tag="st")
                            nc.tensor.matmul(
                                st[:], k_sb[:, h, cs], q_sb[:, h, ts_],
                                start=True, stop=True)
                            nc.scalar.activation(
                                exp_sb[:, c, ts_], st[:], Exp, bias=mbt[:, c:c + 1])
                            nc.tensor.matmul(
                                sum_ps[t][:], ones_col[:], exp_sb[:, c, ts_],
                                start=(c == 0), stop=(c == KC - 1))
                    srow = attn.tile([1, Q], f32, tag="srow")
                    for t in range(NT):
                        nc.vector.tensor_copy(
                            srow[0:1, t * FT:(t + 1) * FT], sum_ps[t][:])
                    rrow = attn.tile([1, Q], f32, tag="rrow")
                    nc.vector.reciprocal(rrow[:], srow[:])
                    rcp_d = dramp.tile([1, Q], f32, tag="rcpd")
                    nc.sync.dma_start(out=rcp_d[:], in_=rrow[:])
                    rb_sb = attn.tile([128, Q], f32, tag="rb")
                    nc.sync.dma_start(out=rb_sb[:], in_=rcp_d[0:1, :].to_broadcast([128, Q]))

                    for t in range(NT):
                        ts_ = slice(t * FT, (t + 1) * FT)
                        cp = cpsum.tile([128, FT], f32, tag="cp")
                        for c in range(KC):
                            nc.tensor.matmul(
                                cp[:], v_sb[:, c, hs], exp_sb[:, c, ts_],
                                start=(c == 0), stop=(c == KC - 1))
                        nc.vector.tensor_mul(ctx_sb[:, h, ts_], cp[:], rb_sb[:, ts_])

                    if h == 0:
                        for c in range(KC):
                            p0_sb = attn3.tile([128, Q], f32, tag="p0")
                            nc.vector.tensor_mul(p0_sb[:], exp_sb[:, c, :], rb_sb[:])
                            nc.sync.dma_start(
                                out=p0t[c * DH:(c + 1) * DH, :], in_=p0_sb[:])

            # ---------------- Phase 3: final projection ----------------
            with tc.tile_pool(name=f"fin{rep}", bufs=3) as fin, \
                 tc.tile_pool(name=f"fpsum{rep}", bufs=3, space="PSUM") as fpsum:
                for qc in range(KC):
                    qs = slice(qc * DH, (qc + 1) * DH)
                    for t in range(NT):
                        ts_ = slice(t * FT, (t + 1) * FT)
                        fp = fpsum.tile([128, FT], f32, tag="fp")
                        for h in range(H):
                            nc.tensor.matmul(
                                fp[:], ctx_sb[:, h, qs], wf_sb[:, h, ts_],
                                start=(h == 0), stop=(h == H - 1))
                        of = fin.tile([128, FT], f32, tag="of")
                        nc.vector.tensor_add(of[:], fp[:], bft[:, ts_])
                        nc.sync.dma_start(out=wm[qs, ts_], in_=of[:])

    split_sync_waits(nc)
    return nc


def _get_program(reps=1):
    key = f"nc{reps}"
    if key not in _CACHE:
        _CACHE[key] = _build_program(reps)
    return _CACHE[key]


def _host_prep(query, memory, mask, Wk, bk, Wv, bv, Wq, bq, Wf, bf):
    scale = 1.0 / math.sqrt(DH)
    f32 = np.float32

    def t_bf16(a):
        return np.ascontiguousarray(np.asarray(a, dtype=f32).T).astype(_BF16)

    shared = {
        "wkT": t_bf16(Wk),
        "wvT": t_bf16(Wv),
        "wqT": np.ascontiguousarray(
            np.asarray(Wq, dtype=f32).T * f32(scale)).astype(_BF16),
        "wfT": t_bf16(Wf),
        "bk_pp": np.ascontiguousarray(
            np.asarray(bk, dtype=f32).reshape(H, DH).T),
        "bq_pp": np.ascontiguousarray(
            (np.asarray(bq, dtype=f32) * f32(scale)).reshape(H, DH).T),
        "bv_bc": np.ascontiguousarray(
            np.broadcast_to(np.asarray(bv, dtype=f32), (128, D))),
        "bf_bc": np.ascontiguousarray(
            np.broadcast_to(np.asarray(bf, dtype=f32), (128, D))),
    }
    mask = np.asarray(mask)
    in_maps = []
    for b in range(B):
        mb = np.where(mask[b], f32(-1e30), f32(0.0)).astype(f32)
        in_maps.append({
            **shared,
            "memT": t_bf16(memory[b]),
            "qT": t_bf16(query[b]),
            "mb_pp": np.ascontiguousarray(mb.reshape(KC, DH).T),
        })
    return in_maps


def kernel(query, memory, mask, Wk, bk, Wv, bv, Wq, bq, Wf, bf):
    from concourse.bass_utils import run_bass_kernel_spmd

    nc = _get_program()
    in_maps = _host_prep(query, memory, mask, Wk, bk, Wv, bv, Wq, bq, Wf, bf)
    res = run_bass_kernel_spmd(nc, in_maps, core_ids=list(range(B)))
    wm = np.stack([res.results[b]["wm"] for b in range(B)])
    w0 = np.stack([np.ascontiguousarray(res.results[b]["p0t"].T)
                   for b in range(B)])
    return wm.astype(np.float32), w0.astype(np.float32)


# revision 7
# speedup vs baseline: 516.7161x; 2.1720x over previous
"""Trainium2 Bass kernel for nn_Attention (dense transformer cross-attention).

Strategy: data-parallel over batch (B=8) -> one batch element per NeuronCore.
Per core, everything is computed with zero on-chip transposes by choosing
layouts up front (host pre-transposes activations/weights, which is free):

  K^T_h [dh=128, M]   = Wk-chunk^T . memory^T   (per head h, bias via ACT)
  Q^T_h [dh=128, Q]   = (scaled Wq)-chunk^T . query^T
  V     [M, D]        = memory . Wv^T + bv      (natural layout)
  S^T   [M, Q]        = K^T_h^T-free-slices . Q^T_h    (heads pipelined)
  expS  = ACT Exp with per-partition mask bias (-1e30 -> exact 0), bf16
  sum_q = ones-row matmul over expS (partition-direction sum on PE)
  ctx^T_h [dh, Q]     = V-chunks . expS  (PSUM accum over m-chunks)
  normalization       = 1/sum broadcast via DRAM-bounce DMA, fused into
                        the ctx PSUM->SBUF drain (DVE multiply)
  out   [Q, D]        = ctx^T (as lhsT, heads = contraction chunks) . Wf^T
                        + bf (fused into final drain)

Softmax max-subtraction is skipped: scores are O(1) by construction
(0.02-scale weights), exp is computed in f32 on ACT, so this is exact.

Compute dtype bf16 (f32 PSUM accumulation); inputs converted on host so
device DMA moves half the bytes.
"""

import math

import numpy as np
import ml_dtypes

B = 8
Q = 1024
M = 1024
D = 1024
H = 8
DH = 128
KC = 8  # 128-row contraction chunks per 1024
NT = 2  # 512-wide free tiles per 1024
FT = 512

_BF16 = ml_dtypes.bfloat16
_CACHE = {}


def _build_program(reps=1):
    import concourse.bass as bass
    import concourse.mybir as mybir
    from concourse.tile import TileContext
    from concourse.vector_clock import ScopedClock

    import bass_rust

    f32 = mybir.dt.float32
    bf16 = mybir.dt.bfloat16
    Identity = mybir.ActivationFunctionType.Identity
    Exp = mybir.ActivationFunctionType.Exp

    def split_sync_waits(nc):
        """The walrus in this container accepts only ONE sync-wait per
        instruction; Tile freely attaches several. Move excess waits onto
        same-engine NOPs spliced immediately before the instruction."""
        for fn in nc.m.functions:
            for bb in fn.blocks:
                out = []
                for inst in bb.instructions:
                    si = inst.sync_info
                    if si is not None and si.on_wait is not None and len(si.on_wait) > 1:
                        waits = list(si.on_wait)
                        si.on_wait = waits[-1:]
                        for j, w in enumerate(waits[:-1]):
                            nop = bass_rust.InstNoOp(
                                name=f"{inst.name}_sw{j}", ins=[], outs=[])
                            nop.engine = inst.engine
                            nop.sync_info = mybir.SyncInfo(on_wait=[w], on_update=[])
                            out.append(nop)
                    out.append(inst)
                bb.instructions = out

    nc = bass.Bass()

    memT = nc.declare_dram_parameter("memT", [D, M], bf16, isOutput=False)
    qT = nc.declare_dram_parameter("qT", [D, Q], bf16, isOutput=False)
    wkT = nc.declare_dram_parameter("wkT", [D, D], bf16, isOutput=False)
    wvT = nc.declare_dram_parameter("wvT", [D, D], bf16, isOutput=False)
    wqT = nc.declare_dram_parameter("wqT", [D, D], bf16, isOutput=False)
    wfT = nc.declare_dram_parameter("wfT", [D, D], bf16, isOutput=False)
    bk_pp = nc.declare_dram_parameter("bk_pp", [128, H], f32, isOutput=False)
    bq_pp = nc.declare_dram_parameter("bq_pp", [128, H], f32, isOutput=False)
    mb_pp = nc.declare_dram_parameter("mb_pp", [128, KC], f32, isOutput=False)
    bv_bc = nc.declare_dram_parameter("bv_bc", [128, D], f32, isOutput=False)
    bf_bc = nc.declare_dram_parameter("bf_bc", [128, D], f32, isOutput=False)

    wm = nc.declare_dram_parameter("wm", [Q, D], f32, isOutput=True)
    p0t = nc.declare_dram_parameter("p0t", [M, Q], f32, isOutput=True)

    def chunked(dram_ap):
        # [1024, N] DRAM -> [p=128, c=8, N] access pattern
        return dram_ap.rearrange("(c p) n -> p c n", p=128)

    with TileContext(nc) as tc:
      for rep in range(reps):
        with tc.tile_pool(name=f"const{rep}", bufs=1) as const, \
             tc.tile_pool(name=f"persist{rep}", bufs=1) as persist:
            wf_sb = const.tile([128, KC, D], bf16)
            bkt = const.tile([128, H], f32)
            bqt = const.tile([128, H], f32)
            mbt = const.tile([128, KC], f32)
            bvt = const.tile([128, D], f32)
            bft = const.tile([128, D], f32)
            ones_col = const.tile([128, 1], bf16)

            nc.sync.dma_start(out=wf_sb[:], in_=chunked(wfT[:, :]))
            nc.sync.dma_start(out=bkt[:], in_=bk_pp[:, :])
            nc.sync.dma_start(out=bqt[:], in_=bq_pp[:, :])
            nc.sync.dma_start(out=mbt[:], in_=mb_pp[:, :])
            nc.sync.dma_start(out=bvt[:], in_=bv_bc[:, :])
            nc.sync.dma_start(out=bft[:], in_=bf_bc[:, :])
            nc.vector.memset(ones_col[:], 1.0)

            k_sb = persist.tile([128, H, M], bf16)
            q_sb = persist.tile([128, H, Q], bf16)
            v_sb = persist.tile([128, KC, D], bf16)
            ctx_sb = persist.tile([128, H, Q], bf16)

            # ---------------- Phase 1: projections ----------------
            with tc.tile_pool(name=f"proj{rep}", bufs=1) as proj, \
                 tc.tile_pool(name=f"ppsum{rep}", bufs=4, space="PSUM") as ppsum:
                mem_sb = proj.tile([128, KC, M], bf16)
                qt_sb = proj.tile([128, KC, Q], bf16)
                wk_sb = proj.tile([128, KC, D], bf16)
                wv_sb = proj.tile([128, KC, D], bf16)
                wq_sb = proj.tile([128, KC, D], bf16)
                nc.sync.dma_start(out=mem_sb[:], in_=chunked(memT[:, :]))
                nc.sync.dma_start(out=qt_sb[:], in_=chunked(qT[:, :]))
                nc.sync.dma_start(out=wk_sb[:], in_=chunked(wkT[:, :]))
                nc.sync.dma_start(out=wv_sb[:], in_=chunked(wvT[:, :]))
                nc.sync.dma_start(out=wq_sb[:], in_=chunked(wqT[:, :]))

                for h in range(H):
                    hs = slice(h * DH, (h + 1) * DH)
                    for t in range(NT):
                        ts_ = slice(t * FT, (t + 1) * FT)
                        ps = ppsum.tile([128, FT], f32, tag="pp")
                        for c in range(KC):
                            nc.tensor.matmul(
                                ps[:], wk_sb[:, c, hs], mem_sb[:, c, ts_],
                                start=(c == 0), stop=(c == KC - 1))
                        nc.scalar.activation(
                            k_sb[:, h, ts_], ps[:], Identity, bias=bkt[:, h:h + 1])
                        ps2 = ppsum.tile([128, FT], f32, tag="pp")
                        for c in range(KC):
                            nc.tensor.matmul(
                                ps2[:], wq_sb[:, c, hs], qt_sb[:, c, ts_],
                                start=(c == 0), stop=(c == KC - 1))
                        nc.scalar.activation(
                            q_sb[:, h, ts_], ps2[:], Identity, bias=bqt[:, h:h + 1])

                for mc in range(KC):
                    ms = slice(mc * DH, (mc + 1) * DH)
                    for t in range(NT):
                        ts_ = slice(t * FT, (t + 1) * FT)
                        ps = ppsum.tile([128, FT], f32, tag="pp")
                        for c in range(KC):
                            nc.tensor.matmul(
                                ps[:], mem_sb[:, c, ms], wv_sb[:, c, ts_],
                                start=(c == 0), stop=(c == KC - 1))
                        nc.vector.tensor_add(v_sb[:, mc, ts_], ps[:], bvt[:, ts_])

            # ---------------- Phase 2: attention (per head) ----------------
            with tc.tile_pool(name=f"attn{rep}", bufs=2) as attn, \
                 tc.tile_pool(name=f"attn3{rep}", bufs=3) as attn3, \
                 tc.tile_pool(name=f"dramp{rep}", bufs=2, space="DRAM") as dramp, \
                 tc.tile_pool(name=f"spsum{rep}", bufs=2, space="PSUM") as spsum, \
                 tc.tile_pool(name=f"cpsum{rep}", bufs=3, space="PSUM") as cpsum, \
                 tc.tile_pool(name=f"upsum{rep}", bufs=2, space="PSUM") as upsum:
                for h in range(H):
                    hs = slice(h * DH, (h + 1) * DH)
                    exp_sb = attn.tile([128, KC, Q], bf16, tag="expS")
                    sum_ps = [upsum.tile([1, FT], f32, tag="sum", name=f"sum_h{h}_t{t}")
                              for t in range(NT)]
                    for c in range(KC):
                        cs = slice(c * DH, (c + 1) * DH)
                        for t in range(NT):
                            ts_ = slice(t * FT, (t + 1) * FT)
                            st = spsum.tile([128, FT], f32, tag="st")
                            nc.tensor.matmul(
                                st[:], k_sb[:, h, cs], q_sb[:, h, ts_],
                                start=True, stop=True)
                            nc.scalar.activation(
                                exp_sb[:, c, ts_], st[:], Exp, bias=mbt[:, c:c + 1])
                            nc.tensor.matmul(
                                sum_ps[t][:], ones_col[:], exp_sb[:, c, ts_],
                                start=(c == 0), stop=(c == KC - 1))
                    srow = attn.tile([1, Q], f32, tag="srow")
                    for t in range(NT):
                        nc.vector.tensor_copy(
                            srow[0:1, t * FT:(t + 1) * FT], sum_ps[t][:])
                    rrow = attn.tile([1, Q], f32, tag="rrow")
                    nc.vector.reciprocal(rrow[:], srow[:])
                    rcp_d = dramp.tile([1, Q], f32, tag="rcpd")
                    nc.sync.dma_start(out=rcp_d[:], in_=rrow[:])
                    rb_sb = attn.tile([128, Q], f32, tag="rb")
                    nc.sync.dma_start(out=rb_sb[:], in_=rcp_d[0:1, :].to_broadcast([128, Q]))

                    for t in range(NT):
                        ts_ = slice(t * FT, (t + 1) * FT)
                        cp = cpsum.tile([128, FT], f32, tag="cp")
                        for c in range(KC):
                            nc.tensor.matmul(
                                cp[:], v_sb[:, c, hs], exp_sb[:, c, ts_],
                                start=(c == 0), stop=(c == KC - 1))
                        nc.vector.tensor_mul(ctx_sb[:, h, ts_], cp[:], rb_sb[:, ts_])

                    if h == 0:
                        for c in range(KC):
                            p0_sb = attn3.tile([128, Q], f32, tag="p0")
                            nc.vector.tensor_mul(p0_sb[:], exp_sb[:, c, :], rb_sb[:])
                            nc.sync.dma_start(
                                out=p0t[c * DH:(c + 1) * DH, :], in_=p0_sb[:])

            # ---------------- Phase 3: final projection ----------------
            with tc.tile_pool(name=f"fin{rep}", bufs=3) as fin, \
                 tc.tile_pool(name=f"fpsum{rep}", bufs=3, space="PSUM") as fpsum:
                for qc in range(KC):
                    qs = slice(qc * DH, (qc + 1) * DH)
                    for t in range(NT):
                        ts_ = slice(t * FT, (t + 1) * FT)
                        fp = fpsum.tile([128, FT], f32, tag="fp")
                        for h in range(H):
                            nc.tensor.matmul(
                                fp[:], ctx_sb[:, h, qs], wf_sb[:, h, ts_],
                                start=(h == 0), stop=(h == H - 1))
                        of = fin.tile([128, FT], f32, tag="of")
                        nc.vector.tensor_add(of[:], fp[:], bft[:, ts_])
                        nc.sync.dma_start(out=wm[qs, ts_], in_=of[:])

    split_sync_waits(nc)
    return nc


def _get_program(reps=1):
    key = f"nc{reps}"
    if key not in _CACHE:
        _CACHE[key] = _build_program(reps)
    return _CACHE[key]


def _host_prep(query, memory, mask, Wk, bk, Wv, bv, Wq, bq, Wf, bf):
    scale = 1.0 / math.sqrt(DH)
    f32 = np.float32

    def t_bf16(a):
        return np.ascontiguousarray(np.asarray(a, dtype=f32).T).astype(_BF16)

    shared = {
        "wkT": t_bf16(Wk),
        "wvT": t_bf16(Wv),
        "wqT": np.ascontiguousarray(
            np.asarray(Wq, dtype=f32).T * f32(scale)).astype(_BF16),
        "wfT": t_bf16(Wf),
        "bk_pp": np.ascontiguousarray(
            np.asarray(bk, dtype=f32).reshape(H, DH).T),
        "bq_pp": np.ascontiguousarray(
            (np.asarray(bq, dtype=f32) * f32(scale)).reshape(H, DH).T),
        "bv_bc": np.ascontiguousarray(
            np.broadcast_to(np.asarray(bv, dtype=f32), (128, D))),
        "bf_bc": np.ascontiguousarray(
            np.broadcast_to(np.asarray(bf, dtype=f32), (128, D))),
    }
    mask = np.asarray(mask)
    in_maps = []
    for b in range(B):
        mb = np.where(mask[b], f32(-1e30), f32(0.0)).astype(f32)
        in_maps.append({
            **shared,
            "memT": t_bf16(memory[b]),
            "qT": t_bf16(query[b]),
            "mb_pp": np.ascontiguousarray(mb.reshape(KC, DH).T),
        })
    return in_maps


def kernel(query, memory, mask, Wk, bk, Wv, bv, Wq, bq, Wf, bf):
    from concourse.bass_utils import run_bass_kernel_spmd

    nc = _get_program()
    in_maps = _host_prep(query, memory, mask, Wk, bk, Wv, bv, Wq, bq, Wf, bf)
    res = run_bass_kernel_spmd(nc, in_maps, core_ids=list(range(B)))
    wm = np.stack([res.results[b]["wm"] for b in range(B)])
    w0 = np.stack([np.ascontiguousarray(res.results[b]["p0t"].T)
                   for b in range(B)])
    return wm.astype(np.float32), w0.astype(np.float32)


# revision 22
# speedup vs baseline: 569.8326x; 1.1028x over previous
"""Trainium2 Bass kernel for nn_Attention (dense transformer cross-attention).

Strategy: data-parallel over batch (B=8) -> one batch element per NeuronCore.
Per core, everything is computed with zero on-chip transposes by choosing
layouts up front (host pre-transposes activations/weights, which is free):

  K^T_h [dh=128, M]   = Wk-chunk^T . memory^T   (per head h, bias via ACT)
  Q^T_h [dh=128, Q]   = (scaled Wq)-chunk^T . query^T
  V     [M, D]        = memory . Wv^T + bv      (natural layout)
  S^T   [M, Q]        = K^T_h^T-free-slices . Q^T_h    (heads pipelined)
  expS  = ACT Exp with per-partition mask bias (-1e30 -> exact 0), bf16
  sum_q = ones-row matmul over expS (partition-direction sum on PE)
  ctx^T_h [dh, Q]     = V-chunks . expS  (PSUM accum over m-chunks)
  normalization       = 1/sum broadcast via DRAM-bounce DMA, fused into
                        the ctx PSUM->SBUF drain (DVE multiply)
  out   [Q, D]        = ctx^T (as lhsT, heads = contraction chunks) . Wf^T
                        + bf (fused into final drain)

Softmax max-subtraction is skipped: scores are O(1) by construction
(0.02-scale weights), exp is computed in f32 on ACT, so this is exact.

Compute dtype bf16 (f32 PSUM accumulation); inputs converted on host so
device DMA moves half the bytes.
"""

import math

import numpy as np
import ml_dtypes

B = 8
Q = 1024
M = 1024
D = 1024
H = 8
DH = 128
KC = 8  # 128-row contraction chunks per 1024
NT = 2  # 512-wide free tiles per 1024
FT = 512

_BF16 = ml_dtypes.bfloat16
_CACHE = {}


def _build_program(reps=1, parts='paf', norm=True):
    import concourse.bass as bass
    import concourse.mybir as mybir
    from concourse.tile import TileContext
    from concourse.vector_clock import ScopedClock

    import bass_rust

    f32 = mybir.dt.float32
    bf16 = mybir.dt.bfloat16
    Identity = mybir.ActivationFunctionType.Identity
    Exp = mybir.ActivationFunctionType.Exp

    def split_sync_waits(nc):
        """The walrus in this container accepts only ONE sync-wait per
        instruction; Tile freely attaches several. Move excess waits onto
        same-engine NOPs spliced immediately before the instruction."""
        for fn in nc.m.functions:
            for bb in fn.blocks:
                out = []
                for inst in bb.instructions:
                    si = inst.sync_info
                    if si is not None and si.on_wait is not None and len(si.on_wait) > 1:
                        waits = list(si.on_wait)
                        si.on_wait = waits[-1:]
                        for j, w in enumerate(waits[:-1]):
                            nop = bass_rust.InstNoOp(
                                name=f"{inst.name}_sw{j}", ins=[], outs=[])
                            nop.engine = inst.engine
                            nop.sync_info = mybir.SyncInfo(on_wait=[w], on_update=[])
                            out.append(nop)
                    out.append(inst)
                bb.instructions = out

    nc = bass.Bass()

    memT = nc.declare_dram_parameter("memT", [D, M], bf16, isOutput=False)
    qT = nc.declare_dram_parameter("qT", [D, Q], bf16, isOutput=False)
    wkT = nc.declare_dram_parameter("wkT", [D, D], bf16, isOutput=False)
    wvT = nc.declare_dram_parameter("wvT", [D, D], bf16, isOutput=False)
    wqT = nc.declare_dram_parameter("wqT", [D, D], bf16, isOutput=False)
    wfT = nc.declare_dram_parameter("wfT", [D, D], bf16, isOutput=False)
    bk_pp = nc.declare_dram_parameter("bk_pp", [128, H], f32, isOutput=False)
    bq_pp = nc.declare_dram_parameter("bq_pp", [128, H], f32, isOutput=False)
    mb_pp = nc.declare_dram_parameter("mb_pp", [128, KC], f32, isOutput=False)
    bv_bc = nc.declare_dram_parameter("bv_bc", [128, D], f32, isOutput=False)
    bf_bc = nc.declare_dram_parameter("bf_bc", [128, D], f32, isOutput=False)

    wm = nc.declare_dram_parameter("wm", [Q, D], f32, isOutput=True)
    p0t = nc.declare_dram_parameter("p0t", [M, Q], f32, isOutput=True)

    def chunked(dram_ap):
        # [1024, N] DRAM -> [p=128, c=8, N] access pattern
        return dram_ap.rearrange("(c p) n -> p c n", p=128)

    with TileContext(nc) as tc:
      for rep in range(reps):
        with tc.tile_pool(name=f"const{rep}", bufs=1) as const, \
             tc.tile_pool(name=f"persist{rep}", bufs=1) as persist:
            wf_sb = const.tile([128, KC, D], bf16)
            bkt = const.tile([128, H], f32)
            bqt = const.tile([128, H], f32)
            mbt = const.tile([128, KC], f32)
            bvt = const.tile([128, D], f32)
            bft = const.tile([128, D], f32)
            ones128 = const.tile([128, 128], bf16)

            nc.scalar.dma_start(out=bkt[:], in_=bk_pp[:, :])
            nc.scalar.dma_start(out=bqt[:], in_=bq_pp[:, :])
            nc.scalar.dma_start(out=mbt[:], in_=mb_pp[:, :])
            nc.scalar.dma_start(out=bvt[:], in_=bv_bc[:, :])
            nc.scalar.dma_start(out=bft[:], in_=bf_bc[:, :])
            nc.scalar.dma_start(out=wf_sb[:], in_=chunked(wfT[:, :]))
            nc.vector.memset(ones128[:], 1.0)

            k_sb = persist.tile([128, H, M], bf16)
            q_sb = persist.tile([128, H, Q], bf16)
            v_sb = persist.tile([128, KC, D], bf16)
            ctx_sb = persist.tile([128, H, Q], bf16)

            # ---------------- Phase 1: projections ----------------
            with tc.tile_pool(name=f"proj{rep}", bufs=1) as proj, \
                 tc.tile_pool(name=f"ppsum{rep}", bufs=4, space="PSUM") as ppsum:
                mem_sb = proj.tile([128, KC, M], bf16)
                qt_sb = proj.tile([128, KC, Q], bf16)
                wk_sb = proj.tile([128, KC, D], bf16)
                wv_sb = proj.tile([128, KC, D], bf16)
                wq_sb = proj.tile([128, KC, D], bf16)
                # K-projection inputs stream per chunk pair so PE starts
                # accumulating as data lands; Q inputs next, Wv last.
                for c in range(KC):
                    nc.sync.dma_start(out=wk_sb[:, c, :], in_=chunked(wkT[:, :])[:, c, :])
                    nc.sync.dma_start(out=mem_sb[:, c, :], in_=chunked(memT[:, :])[:, c, :])
                nc.sync.dma_start(out=qt_sb[:], in_=chunked(qT[:, :]))
                nc.sync.dma_start(out=wq_sb[:], in_=chunked(wqT[:, :]))
                nc.sync.dma_start(out=wv_sb[:], in_=chunked(wvT[:, :]))

                for h in range(H):
                    hs = slice(h * DH, (h + 1) * DH)
                    ps = ppsum.tile([128, Q], f32, tag="pp")
                    for c in range(KC):
                        for t in range(NT):
                            ts_ = slice(t * FT, (t + 1) * FT)
                            nc.tensor.matmul(
                                ps[:, ts_], wk_sb[:, c, hs], mem_sb[:, c, ts_],
                                start=(c == 0), stop=(c == KC - 1))
                    nc.scalar.activation(
                        k_sb[:, h, :], ps[:], Identity, bias=bkt[:, h:h + 1])
                for h in range(H):
                    hs = slice(h * DH, (h + 1) * DH)
                    ps2 = ppsum.tile([128, Q], f32, tag="pp")
                    for c in range(KC):
                        for t in range(NT):
                            ts_ = slice(t * FT, (t + 1) * FT)
                            nc.tensor.matmul(
                                ps2[:, ts_], wq_sb[:, c, hs], qt_sb[:, c, ts_],
                                start=(c == 0), stop=(c == KC - 1))
                    nc.scalar.activation(
                        q_sb[:, h, :], ps2[:], Identity, bias=bqt[:, h:h + 1])

                for mc in range(KC):
                    ms = slice(mc * DH, (mc + 1) * DH)
                    ps = ppsum.tile([128, D], f32, tag="pp")
                    for c in range(KC):
                        for t in range(NT):
                            ts_ = slice(t * FT, (t + 1) * FT)
                            nc.tensor.matmul(
                                ps[:, ts_], mem_sb[:, c, ms], wv_sb[:, c, ts_],
                                start=(c == 0), stop=(c == KC - 1))
                    nc.vector.tensor_add(v_sb[:, mc, :], ps[:], bvt[:])

            # ---------------- Phase 2: attention (per head) ----------------
            if 'a' not in parts:
                continue
            with tc.tile_pool(name=f"attn{rep}", bufs=2) as attn, \
                 tc.tile_pool(name=f"attn3{rep}", bufs=3) as attn3, \
                 tc.tile_pool(name=f"dramp{rep}", bufs=2, space="DRAM") as dramp, \
                 tc.tile_pool(name=f"spsum{rep}", bufs=2, space="PSUM") as spsum, \
                 tc.tile_pool(name=f"cpsum{rep}", bufs=2, space="PSUM") as cpsum, \
                 tc.tile_pool(name=f"upsum{rep}", bufs=2, space="PSUM") as upsum:
                for h in range(H):
                    hs = slice(h * DH, (h + 1) * DH)
                    exp_sb = attn.tile([128, KC, Q], bf16, tag="expS", bufs=3)
                    for c in range(KC):
                        cs = slice(c * DH, (c + 1) * DH)
                        st = spsum.tile([128, Q], f32, tag="st")
                        for t in range(NT):
                            ts_ = slice(t * FT, (t + 1) * FT)
                            nc.tensor.matmul(
                                st[:, ts_], k_sb[:, h, cs], q_sb[:, h, ts_],
                                start=True, stop=True)
                        nc.scalar.activation(
                            exp_sb[:, c, :], st[:], Exp, bias=mbt[:, c:c + 1])
                    if norm:
                        # running partial sum over m-chunks on DVE (bf16), then
                        # a ones[128,128]-stationary matmul: every output
                        # partition gets the cross-partition sum, i.e. the
                        # broadcast comes free. Reciprocal drains it to SBUF.
                        acc = attn.tile([128, Q], bf16, tag="acc")
                        nc.vector.tensor_add(acc[:], exp_sb[:, 0, :], exp_sb[:, 1, :])
                        for c in range(2, KC):
                            nc.vector.tensor_add(acc[:], acc[:], exp_sb[:, c, :])
                        rb_sb = attn.tile([128, Q], f32, tag="rb")
                        for t in range(NT):
                            ts_ = slice(t * FT, (t + 1) * FT)
                            sum_bc = upsum.tile([128, FT], f32, tag="sum",
                                                name=f"sum_h{h}_t{t}")
                            nc.tensor.matmul(
                                sum_bc[:], ones128[:], acc[:, ts_],
                                start=True, stop=True)
                            nc.vector.reciprocal(rb_sb[:, ts_], sum_bc[:])

                    for t in range(NT):
                        ts_ = slice(t * FT, (t + 1) * FT)
                        cp = cpsum.tile([128, FT], f32, tag="cp")
                        for c in range(KC):
                            nc.tensor.matmul(
                                cp[:], v_sb[:, c, hs], exp_sb[:, c, ts_],
                                start=(c == 0), stop=(c == KC - 1))
                        if norm:
                            nc.vector.tensor_mul(ctx_sb[:, h, ts_], cp[:], rb_sb[:, ts_])
                        else:
                            nc.vector.tensor_copy(ctx_sb[:, h, ts_], cp[:])

                    if h == 0 and norm:
                        for c in range(KC):
                            p0_sb = attn3.tile([128, Q], f32, tag="p0")
                            nc.gpsimd.tensor_mul(p0_sb[:], exp_sb[:, c, :], rb_sb[:])
                            nc.sync.dma_start(
                                out=p0t[c * DH:(c + 1) * DH, :], in_=p0_sb[:])

            # ---------------- Phase 3: final projection ----------------
            if 'f' not in parts:
                continue
            with tc.tile_pool(name=f"fin{rep}", bufs=3) as fin, \
                 tc.tile_pool(name=f"fpsum{rep}", bufs=3, space="PSUM") as fpsum:
                for qc in range(KC):
                    qs = slice(qc * DH, (qc + 1) * DH)
                    for t in range(NT):
                        ts_ = slice(t * FT, (t + 1) * FT)
                        fp = fpsum.tile([128, FT], f32, tag="fp")
                        for h in range(H):
                            nc.tensor.matmul(
                                fp[:], ctx_sb[:, h, qs], wf_sb[:, h, ts_],
                                start=(h == 0), stop=(h == H - 1))
                        of = fin.tile([128, FT], f32, tag="of")
                        nc.vector.tensor_add(of[:], fp[:], bft[:, ts_])
                        eng = nc.scalar if (qc + t) % 2 == 0 else nc.sync
                        eng.dma_start(out=wm[qs, ts_], in_=of[:])

    split_sync_waits(nc)
    return nc


def _get_program(reps=1, parts='paf', norm=True):
    key = f"nc{reps}_{parts}_{norm}"
    if key not in _CACHE:
        _CACHE[key] = _build_program(reps, parts, norm)
    return _CACHE[key]


def _host_prep(query, memory, mask, Wk, bk, Wv, bv, Wq, bq, Wf, bf):
    scale = 1.0 / math.sqrt(DH)
    f32 = np.float32

    def t_bf16(a):
        return np.ascontiguousarray(np.asarray(a, dtype=f32).T).astype(_BF16)

    shared = {
        "wkT": t_bf16(Wk),
        "wvT": t_bf16(Wv),
        "wqT": np.ascontiguousarray(
            np.asarray(Wq, dtype=f32).T * f32(scale)).astype(_BF16),
        "wfT": t_bf16(Wf),
        "bk_pp": np.ascontiguousarray(
            np.asarray(bk, dtype=f32).reshape(H, DH).T),
        "bq_pp": np.ascontiguousarray(
            (np.asarray(bq, dtype=f32) * f32(scale)).reshape(H, DH).T),
        "bv_bc": np.ascontiguousarray(
            np.broadcast_to(np.asarray(bv, dtype=f32), (128, D))),
        "bf_bc": np.ascontiguousarray(
            np.broadcast_to(np.asarray(bf, dtype=f32), (128, D))),
    }
    mask = np.asarray(mask)
    in_maps = []
    for b in range(B):
        mb = np.where(mask[b], f32(-1e30), f32(0.0)).astype(f32)
        in_maps.append({
            **shared,
            "memT": t_bf16(memory[b]),
            "qT": t_bf16(query[b]),
            "mb_pp": np.ascontiguousarray(mb.reshape(KC, DH).T),
        })
    return in_maps


def kernel(query, memory, mask, Wk, bk, Wv, bv, Wq, bq, Wf, bf):
    from concourse.bass_utils import run_bass_kernel_spmd

    nc = _get_program()
    in_maps = _host_prep(query, memory, mask, Wk, bk, Wv, bv, Wq, bq, Wf, bf)
    res = run_bass_kernel_spmd(nc, in_maps, core_ids=list(range(B)))
    wm = np.stack([res.results[b]["wm"] for b in range(B)])
    w0 = np.stack([np.ascontiguousarray(res.results[b]["p0t"].T)
                   for b in range(B)])
    return wm.astype(np.float32), w0.astype(np.float32)
